# revision 5
# baseline (speedup 1.0000x reference)
"""AtomicConv (gnn_message_passing) Trainium2 kernel.

out[v, t*K+k] = sum_{e: dst[e]=v, feat[src[e]]=t} exp(-scal_k*(d_e-mu_k)^2) * win(d_e)
with win(d) = 0.5*(cos(pi*d/cutoff)+1) for d <= cutoff.

Strategy (8 NeuronCores, edge segments dealt across cores):
  * Host: sort edges by (dst, src_type) -> contiguous (v,t) segments; deal
    segments round-robin by length over 64 streams (8 cores x 8 gpsimd-group
    streams).  Within a stream, segments of equal length m are adjacent, so a
    segment sum is a fixed-stride tensor_reduce([128, c, m]) - no scatter,
    gather or scan on device.
  * Device layout: partition p = (group g = p//16, filter k = p%16).  The
    per-edge distance stream of group g is broadcast to its 16 partitions with
    a 0/1 indicator matmul on the (otherwise idle) tensor engine -> PSUM.
  * ScalarE computes Square(d - mu_k) (per-partition bias) then
    Exp(-scal_k * sq + ln(0.5)); the cosine window 0.5*(1+sin(pi*d/c + pi/2))
    is computed once per slot in a 16x-smaller "blocked" layout and broadcast
    through the tensor engine too; VectorE fuses he = (win_sin + 1) * gauss
    and does the bucketed reduces.
  * Host unpermutes the dense per-stream row blocks into the (V, T*K) output.

The kernel is self-contained: shapes/sharding hardcoded for the
V=100000, E=3200000, K=16, T=4 problem (but layout is data-derived at call
time, so any same-shape input works).
"""

import math
import os
import sys

import numpy as np

sys.path.insert(0, "/opt/trn_rl_repo")

V, E, K, T = 100000, 3200000, 16, 4
NCORES = 8
NGROUP = 8  # streams per core == gpsimd groups
NSTREAM = NCORES * NGROUP
MAXSEG = 64  # segments longer than this are split into chunks
PIECE = 512  # max moving free dim for fp32 matmul

PAD_D = None  # set to cutoff at runtime (win(cutoff) == 0)

LAST_RESULTS = {}  # test harness introspection


def _host_layout(feat, distances, src, dst, ftu):
    """Sort + deal edges; build device input arrays and unpermute metadata."""
    feat = np.asarray(feat, np.float32).reshape(-1)
    d = np.asarray(distances, np.float32).reshape(-1)
    src = np.asarray(src, np.int64).reshape(-1)
    dst = np.asarray(dst, np.int64).reshape(-1)
    ftu = np.asarray(ftu, np.float32).reshape(-1)
    nE = d.shape[0]
    assert ftu.shape[0] == T

    # src type index by value match against features_to_use (general one-hot)
    fs = feat[src]
    match = fs[:, None] == ftu[None, :]
    t_src = np.argmax(match, axis=1).astype(np.int64)
    valid = match.any(axis=1)

    key = dst * T + t_src
    if not valid.all():
        key = key[valid]
        d = d[valid]
    order = np.argsort(key, kind="stable")
    d_s = d[order]
    key_s = key[order]

    uk, uidx, ucnt = np.unique(key_s, return_index=True, return_counts=True)
    if ucnt.max(initial=0) > MAXSEG:
        nch = -(-ucnt // MAXSEG)
        seg_key = np.repeat(uk, nch)
        seg_len = np.full(int(nch.sum()), MAXSEG, np.int64)
        # trailing chunk lengths
        ends = np.cumsum(nch) - 1
        seg_len[ends] = ucnt - (nch - 1) * MAXSEG
        seg_start = np.concatenate([[0], np.cumsum(seg_len)[:-1]])
    else:
        seg_key, seg_start, seg_len = uk, uidx, ucnt.astype(np.int64)
    nseg = len(seg_key)

    # deal segments round-robin by length
    sorder = np.argsort(seg_len, kind="stable")
    slen_sorted = seg_len[sorder]
    lens, lcnt = np.unique(slen_sorted, return_counts=True)
    caps = -(-lcnt // NSTREAM)  # per-stream per-bucket segment capacity
    slot_off = np.concatenate([[0], np.cumsum(caps * lens)]).astype(np.int64)
    row_off = np.concatenate([[0], np.cumsum(caps)]).astype(np.int64)
    S_need = int(slot_off[-1])
    ROWS = int(row_off[-1])

    bstart = np.concatenate([[0], np.cumsum(lcnt)])
    rank = np.arange(nseg) - np.repeat(bstart[:-1], lcnt)
    b_of = np.repeat(np.arange(len(lens)), lcnt)
    strm = rank % NSTREAM
    sidx = rank // NSTREAM
    slotbase = slot_off[b_of] + sidx * lens[b_of]
    rowpos = row_off[b_of] + sidx
    inv = np.empty(nseg, np.int64)
    inv[sorder] = np.arange(nseg)
    strm_o = strm[inv]
    slotbase_o = slotbase[inv]
    rowpos_o = rowpos[inv]

    # per-edge slot placement
    e_seg = np.repeat(np.arange(nseg), seg_len)
    e_off = np.arange(len(d_s)) - np.repeat(seg_start, seg_len)
    e_strm = strm_o[e_seg]
    e_slot = slotbase_o[e_seg] + e_off

    S = -(-S_need // 16) * 16
    pad_d = float(PAD_D)
    d_all = np.full((NSTREAM, S), pad_d, np.float32)
    d_all[e_strm, e_slot] = d_s

    # piece list: (slot offset, segments, m, row offset)
    pieces = []
    for b in range(len(lens)):
        m = int(lens[b])
        cap = int(caps[b])
        o = int(slot_off[b])
        ro = int(row_off[b])
        left = cap
        while left > 0:
            c = min(PIECE // m, left)
            pieces.append((o, c, m, ro))
            o += c * m
            ro += c
            left -= c
    npieces = len(pieces)
    S16 = -(-npieces // 16) * PIECE

    # blocked layout (piece p -> partition j = p%16, slot16 = (p//16)*PIECE)
    d_all3 = d_all.reshape(NCORES, NGROUP, S)
    d_b = np.full((NCORES, NGROUP, 16, S16), pad_d, np.float32)
    for p, (o, c, m, ro) in enumerate(pieces):
        j, s0 = p % 16, (p // 16) * PIECE
        psz = c * m
        d_b[:, :, j, s0 : s0 + psz] = d_all3[:, :, o : o + psz]
    d_b = d_b.reshape(NCORES, 128, S16)

    return dict(
        d_all=d_all, d_b=d_b, pieces=pieces, S=S, S16=S16, ROWS=ROWS,
        seg_key=seg_key, strm_o=strm_o, rowpos_o=rowpos_o,
    )


def _install_trace_shim(bass_utils):
    """Wire the NTFF profile hook that this image's antenv lacks, and make
    artifact upload local-only."""
    import types
    import contextlib
    import ctypes

    if "antenv.axon_hooks" not in sys.modules:
        mod = types.ModuleType("antenv.axon_hooks")
        mod._hook = None
        def set_axon_ntff_profile_hook(h):
            mod._hook = h
        def get_axon_ntff_profile_hook():
            return mod._hook
        mod.set_axon_ntff_profile_hook = set_axon_ntff_profile_hook
        mod.get_axon_ntff_profile_hook = get_axon_ntff_profile_hook
        sys.modules["antenv.axon_hooks"] = mod
        import antenv
        antenv.axon_hooks = mod

        so_path = "/opt/axon/libaxon_pjrt.so"
        if os.path.exists(so_path):
            lib = ctypes.CDLL(so_path)
            if hasattr(lib, "axon_start_nrt_profile"):
                lib.axon_start_nrt_profile.argtypes = [
                    ctypes.POINTER(ctypes.c_int64), ctypes.c_size_t]
                lib.axon_start_nrt_profile.restype = ctypes.c_int64
                lib.axon_stop_nrt_profile.argtypes = [ctypes.c_char_p]
                lib.axon_stop_nrt_profile.restype = ctypes.c_int64

                @contextlib.contextmanager
                def _hook(output_dir, device_ids):
                    import jax
                    jax.devices()
                    if device_ids:
                        ids = (ctypes.c_int64 * len(device_ids))(*device_ids)
                        rc = lib.axon_start_nrt_profile(ids, len(device_ids))
                    else:
                        rc = lib.axon_start_nrt_profile(None, 0)
                    if rc != 0:
                        raise RuntimeError(f"axon_start_nrt_profile rc={rc}")
                    try:
                        yield
                    finally:
                        n = lib.axon_stop_nrt_profile(str(output_dir).encode())
                        print(f"profile: {n} ntff file(s) -> {output_dir}",
                              file=sys.stderr)

                set_axon_ntff_profile_hook(_hook)

    bass_utils.upload_artifacts = lambda tmpdir: f"local://{tmpdir}"


_NC_CACHE = {}


def _build_nc(S, S16, ROWS, pieces, probe=False):
    import concourse.bacc as bacc
    import concourse.tile as tile
    from concourse import mybir
    from contextlib import ExitStack

    cache_key = (S, S16, ROWS, tuple(pieces), probe)
    if cache_key in _NC_CACHE:
        return _NC_CACHE[cache_key]

    f32 = mybir.dt.float32
    AF = mybir.ActivationFunctionType
    ALU = mybir.AluOpType

    nc = bacc.Bacc("TRN2", target_bir_lowering=False, debug=False,
                   num_devices=NCORES)
    d_c_t = nc.dram_tensor("d_c", (NGROUP, S), f32, kind="ExternalInput")
    d_b_t = nc.dram_tensor("d_b", (128, S16), f32, kind="ExternalInput")
    vec_t = nc.dram_tensor("vecs", (128, 5), f32, kind="ExternalInput")
    out_t = nc.dram_tensor("out", (NGROUP, 16, ROWS), f32, kind="ExternalOutput")

    ind_d = np.zeros((NGROUP, 128), np.float32)
    ind_d[np.arange(128) // 16, np.arange(128)] = 1.0
    ind_d_t = nc.inline_tensor(ind_d, "ind_d")
    ind_w = np.zeros((16, 128, 128), np.float32)
    for j in range(16):
        ind_w[j, (np.arange(128) // 16) * 16 + j, np.arange(128)] = 1.0
    ind_w_t = nc.inline_tensor(
        np.ascontiguousarray(ind_w.transpose(1, 0, 2)).reshape(128, 16 * 128),
        "ind_w")

    with tile.TileContext(nc) as tc, ExitStack() as ctx:
        cpool = ctx.enter_context(tc.tile_pool(name="consts", bufs=1))
        lhsT_d = cpool.tile([NGROUP, 128], f32)
        nc.sync.dma_start(lhsT_d[:], ind_d_t.ap())
        lhsT_w = cpool.tile([128, 16 * 128], f32)
        nc.sync.dma_start(lhsT_w[:], ind_w_t.ap())
        vec = cpool.tile([128, 5], f32)
        nc.sync.dma_start(vec[:], vec_t.ap())
        dbt = cpool.tile([128, S16], f32)
        nc.sync.dma_start(dbt[:], d_b_t.ap())
        winb = cpool.tile([128, S16], f32)
        # win_sin = sin(pi/c * d + pi/2); win = 0.5*(1+win_sin)
        nc.scalar.activation(winb[:], dbt[:], AF.Sin,
                             bias=vec[:, 3:4], scale=vec[:, 2:3])
        R = cpool.tile([128, ROWS], f32)

        dcp = ctx.enter_context(tc.tile_pool(name="dc", bufs=3))
        pdp = ctx.enter_context(tc.tile_pool(name="pd", bufs=2, space="PSUM"))
        pwp = ctx.enter_context(tc.tile_pool(name="pw", bufs=2, space="PSUM"))
        sqp = ctx.enter_context(tc.tile_pool(name="sq", bufs=3))
        g2p = ctx.enter_context(tc.tile_pool(name="g2", bufs=3))
        hep = ctx.enter_context(tc.tile_pool(name="he", bufs=3))

        for p, (o, c, m, ro) in enumerate(pieces):
            psz = c * m
            j, s0 = p % 16, (p // 16) * PIECE
            dc = dcp.tile([NGROUP, PIECE], f32, tag="dc")
            nc.sync.dma_start(dc[:, :psz], d_c_t.ap()[:, o : o + psz])
            pd = pdp.tile([128, PIECE], f32, tag="pd")
            nc.tensor.matmul(pd[:, :psz], lhsT_d[:], dc[:, :psz],
                             start=True, stop=True)
            sq = sqp.tile([128, PIECE], f32, tag="sq")
            nc.scalar.activation(sq[:, :psz], pd[:, :psz], AF.Square,
                                 bias=vec[:, 0:1], scale=1.0)
            g2 = g2p.tile([128, PIECE], f32, tag="g2")
            nc.scalar.activation(g2[:, :psz], sq[:, :psz], AF.Exp,
                                 bias=vec[:, 4:5], scale=vec[:, 1:2])
            pw = pwp.tile([128, PIECE], f32, tag="pw")
            nc.tensor.matmul(pw[:, :psz], lhsT_w[:, j * 128 : (j + 1) * 128],
                             winb[:, s0 : s0 + psz], start=True, stop=True)
            he = hep.tile([128, PIECE], f32, tag="he")
            nc.vector.scalar_tensor_tensor(he[:, :psz], pw[:, :psz], 1.0,
                                           g2[:, :psz], op0=ALU.add,
                                           op1=ALU.mult)
            red_in = he[:, :psz].rearrange("q (c m) -> q c m", m=m)
            nc.vector.tensor_reduce(R[:, ro : ro + c], red_in,
                                    axis=mybir.AxisListType.X, op=ALU.add)

        for g in range(NGROUP):
            nc.sync.dma_start(out_t.ap()[g], R[g * 16 : (g + 1) * 16, :])

        if probe:
            import concourse.bass as bass  # noqa
            ppool = ctx.enter_context(tc.tile_pool(name="probe", bufs=1))
            pdat = ppool.tile([128, 1024], f32)
            pidx = ppool.tile([128, 128], mybir.dt.int16)
            pout = ppool.tile([128, 2048], f32)
            nc.vector.memset(pdat[:], 1.0)
            nc.vector.memset(pidx[:].bitcast(f32), 0.0)
            with nc.named_scope("probe_apgather_512"):
                nc.gpsimd.ap_gather(pout[:, :512].rearrange("p (i d) -> p i d", d=1),
                                    pdat[:].rearrange("p (e d) -> p e d", d=1),
                                    pidx[:, :32], channels=128,
                                    num_elems=1024, d=1, num_idxs=512)
            with nc.named_scope("probe_apgather_2048"):
                nc.gpsimd.ap_gather(pout[:, :2048].rearrange("p (i d) -> p i d", d=1),
                                    pdat[:].rearrange("p (e d) -> p e d", d=1),
                                    pidx[:, :128], channels=128,
                                    num_elems=1024, d=1, num_idxs=2048)
            with nc.named_scope("probe_apgather_d16"):
                nc.gpsimd.ap_gather(pout[:, :2048].rearrange("p (i d) -> p i d", d=16),
                                    pdat[:].rearrange("p (e d) -> p e d", d=16),
                                    pidx[:, :8], channels=128,
                                    num_elems=64, d=16, num_idxs=128)

    nc.compile()
    _NC_CACHE[cache_key] = nc
    return nc


def kernel(**inputs):
    global PAD_D
    feat = np.asarray(inputs["feat"], np.float32)
    distances = np.asarray(inputs["distances"], np.float32)
    src = np.asarray(inputs["src"])
    dst = np.asarray(inputs["dst"])
    cutoffs = np.asarray(inputs["interaction_cutoffs"], np.float32)
    mu = np.asarray(inputs["rbf_kernel_means"], np.float32)
    scal = np.asarray(inputs["rbf_kernel_scaling"], np.float32)
    ftu = np.asarray(inputs["features_to_use"], np.float32)

    assert np.all(cutoffs == cutoffs[0]), "per-k cutoffs unsupported"
    cutoff = float(cutoffs[0])
    PAD_D = cutoff  # win(cutoff) == 0 kills padding contributions

    lay = _host_layout(feat, distances, src, dst, ftu)
    S, S16, ROWS, pieces = lay["S"], lay["S16"], lay["ROWS"], lay["pieces"]

    kk = np.arange(128) % 16
    vecs = np.stack([
        -mu[kk],                                    # Square bias
        -scal[kk],                                  # Exp scale
        np.full(128, -math.pi / cutoff, np.float32),  # Sin scale (cos(x)=sin(pi/2-x))
        np.full(128, math.pi / 2, np.float32),       # Sin bias
        np.full(128, math.log(0.5), np.float32),     # Exp bias
    ], axis=1).astype(np.float32)

    probe = bool(int(os.environ.get("KERNEL_PROBE", "0")))
    trace = bool(int(os.environ.get("KERNEL_TRACE", "0")))
    nc = _build_nc(S, S16, ROWS, pieces, probe=probe)

    from concourse import bass_utils
    if trace:
        _install_trace_shim(bass_utils)
    d_all3 = lay["d_all"].reshape(NCORES, NGROUP, S)
    in_maps = [
        {"d_c": np.ascontiguousarray(d_all3[c]),
         "d_b": np.ascontiguousarray(lay["d_b"][c]),
         "vecs": vecs}
        for c in range(NCORES)
    ]
    res = bass_utils.run_bass_kernel_spmd(
        nc, in_maps, core_ids=list(range(NCORES)), trace=trace,
        trace_cores=list(range(NCORES)) if trace else None,
    )
    LAST_RESULTS["res"] = res

    # gather/unshard: dev[core][g][k][row] -> out[v, t*K+k]
    dev = np.stack([r["out"] for r in res.results])  # (8, NGROUP, 16, ROWS)
    rows_all = dev.transpose(0, 1, 3, 2).reshape(NSTREAM, ROWS, K)
    seg_rows = rows_all[lay["strm_o"], lay["rowpos_o"]]  # (nseg, K)
    out = np.zeros((V * T, K), np.float32)
    seg_key = lay["seg_key"]
    if len(np.unique(seg_key)) == len(seg_key):
        out[seg_key] = seg_rows
    else:
        np.add.at(out, seg_key, seg_rows)
    return out.reshape(V, T * K).astype(np.float32)


if __name__ == "__main__":
    # smoke test with tiny random data through the same code paths
    rng = np.random.default_rng(0)
    nE, nV = 5000, 300
    feat = rng.integers(0, T, (nV, 1)).astype(np.float32)
    inputs = dict(
        feat=feat,
        distances=(rng.random((nE, 1)) * 12.0).astype(np.float32),
        src=rng.integers(0, nV, nE).astype(np.int32),
        dst=rng.integers(0, nV, nE).astype(np.int32),
        interaction_cutoffs=np.full(K, 12.0, np.float32),
        rbf_kernel_means=np.linspace(0, 12, K).astype(np.float32),
        rbf_kernel_scaling=np.ones(K, np.float32),
        features_to_use=np.arange(T, dtype=np.float32),
    )
    print(kernel(**inputs).sum())


# revision 7
# speedup vs baseline: 2.3936x; 2.3936x over previous
"""AtomicConv (gnn_message_passing) Trainium2 kernel.

out[v, t*K+k] = sum_{e: dst[e]=v, feat[src[e]]=t} exp(-scal_k*(d_e-mu_k)^2) * win(d_e)
with win(d) = 0.5*(cos(pi*d/cutoff)+1) for d <= cutoff.

Strategy (8 NeuronCores, edge segments dealt across cores):
  * Host: sort edges by (dst, src_type) -> contiguous (v,t) segments; deal
    segments round-robin by length over 64 streams (8 cores x 8 gpsimd-group
    streams).  Within a stream, segments of equal length m are adjacent, so a
    segment sum is a fixed-stride tensor_reduce([128, c, m]) - no scatter,
    gather or scan on device.
  * Device layout: partition p = (group g = p//16, filter k = p%16).  The
    per-edge distance stream of group g is broadcast to its 16 partitions with
    a 0/1 indicator matmul on the (otherwise idle) tensor engine -> PSUM.
  * ScalarE computes Square(d - mu_k) (per-partition bias) then
    Exp(-scal_k * sq + ln(0.5)); the cosine window 0.5*(1+sin(pi*d/c + pi/2))
    is computed once per slot in a 16x-smaller "blocked" layout and broadcast
    through the tensor engine too; VectorE fuses he = (win_sin + 1) * gauss
    and does the bucketed reduces.
  * Host unpermutes the dense per-stream row blocks into the (V, T*K) output.

The kernel is self-contained: shapes/sharding hardcoded for the
V=100000, E=3200000, K=16, T=4 problem (but layout is data-derived at call
time, so any same-shape input works).
"""

import math
import os
import sys

import numpy as np

sys.path.insert(0, "/opt/trn_rl_repo")

V, E, K, T = 100000, 3200000, 16, 4
NCORES = 8
NGROUP = 8  # streams per core == gpsimd groups
NSTREAM = NCORES * NGROUP
MAXSEG = 64  # segments longer than this are split into chunks
PIECE = 1024  # moving free dim (bf16 matmul max; 2 PSUM banks fp32)

PAD_D = None  # set to cutoff at runtime (win(cutoff) == 0)

LAST_RESULTS = {}  # test harness introspection


def _host_layout(feat, distances, src, dst, ftu):
    """Sort + deal edges; build device input arrays and unpermute metadata."""
    feat = np.asarray(feat, np.float32).reshape(-1)
    d = np.asarray(distances, np.float32).reshape(-1)
    src = np.asarray(src, np.int64).reshape(-1)
    dst = np.asarray(dst, np.int64).reshape(-1)
    ftu = np.asarray(ftu, np.float32).reshape(-1)
    nE = d.shape[0]
    assert ftu.shape[0] == T

    # src type index by value match against features_to_use (general one-hot)
    fs = feat[src]
    match = fs[:, None] == ftu[None, :]
    t_src = np.argmax(match, axis=1).astype(np.int64)
    valid = match.any(axis=1)

    key = dst * T + t_src
    if not valid.all():
        key = key[valid]
        d = d[valid]
    order = np.argsort(key, kind="stable")
    d_s = d[order]
    key_s = key[order]

    uk, uidx, ucnt = np.unique(key_s, return_index=True, return_counts=True)
    if ucnt.max(initial=0) > MAXSEG:
        nch = -(-ucnt // MAXSEG)
        seg_key = np.repeat(uk, nch)
        seg_len = np.full(int(nch.sum()), MAXSEG, np.int64)
        # trailing chunk lengths
        ends = np.cumsum(nch) - 1
        seg_len[ends] = ucnt - (nch - 1) * MAXSEG
        seg_start = np.concatenate([[0], np.cumsum(seg_len)[:-1]])
    else:
        seg_key, seg_start, seg_len = uk, uidx, ucnt.astype(np.int64)
    nseg = len(seg_key)

    # deal segments round-robin by length
    sorder = np.argsort(seg_len, kind="stable")
    slen_sorted = seg_len[sorder]
    lens, lcnt = np.unique(slen_sorted, return_counts=True)
    caps = -(-lcnt // NSTREAM)  # per-stream per-bucket segment capacity
    slot_off = np.concatenate([[0], np.cumsum(caps * lens)]).astype(np.int64)
    row_off = np.concatenate([[0], np.cumsum(caps)]).astype(np.int64)
    S_need = int(slot_off[-1])
    ROWS = int(row_off[-1])

    bstart = np.concatenate([[0], np.cumsum(lcnt)])
    rank = np.arange(nseg) - np.repeat(bstart[:-1], lcnt)
    b_of = np.repeat(np.arange(len(lens)), lcnt)
    strm = rank % NSTREAM
    sidx = rank // NSTREAM
    slotbase = slot_off[b_of] + sidx * lens[b_of]
    rowpos = row_off[b_of] + sidx
    inv = np.empty(nseg, np.int64)
    inv[sorder] = np.arange(nseg)
    strm_o = strm[inv]
    slotbase_o = slotbase[inv]
    rowpos_o = rowpos[inv]

    # per-edge slot placement
    e_seg = np.repeat(np.arange(nseg), seg_len)
    e_off = np.arange(len(d_s)) - np.repeat(seg_start, seg_len)
    e_strm = strm_o[e_seg]
    e_slot = slotbase_o[e_seg] + e_off

    S = -(-S_need // 16) * 16
    pad_d = float(PAD_D)
    d_all = np.full((NSTREAM, S), pad_d, np.float32)
    d_all[e_strm, e_slot] = d_s
    # 3-way bf16 split: d == dh + dm + dl to ~1e-7 abs
    import ml_dtypes
    bf16 = ml_dtypes.bfloat16
    dh = d_all.astype(bf16)
    r1 = d_all - dh.astype(np.float32)
    dm = r1.astype(bf16)
    r2 = r1 - dm.astype(np.float32)
    dl = r2.astype(bf16)
    # (NCORES, NGROUP, 3, S) -> rows g*3+part
    d_parts = np.stack([dh, dm, dl], axis=1).reshape(NSTREAM, 3, S)
    d_parts = d_parts.reshape(NCORES, NGROUP * 3, S)

    # piece list: (slot offset, segments, m, row offset)
    pieces = []
    for b in range(len(lens)):
        m = int(lens[b])
        cap = int(caps[b])
        o = int(slot_off[b])
        ro = int(row_off[b])
        left = cap
        while left > 0:
            c = min(PIECE // m, left)
            pieces.append((o, c, m, ro))
            o += c * m
            ro += c
            left -= c
    npieces = len(pieces)
    S16 = -(-npieces // 16) * PIECE

    # blocked layout (piece p -> partition j = p%16, slot16 = (p//16)*PIECE)
    d_all3 = d_all.reshape(NCORES, NGROUP, S)
    d_b = np.full((NCORES, NGROUP, 16, S16), pad_d, np.float32)
    for p, (o, c, m, ro) in enumerate(pieces):
        j, s0 = p % 16, (p // 16) * PIECE
        psz = c * m
        d_b[:, :, j, s0 : s0 + psz] = d_all3[:, :, o : o + psz]
    d_b = d_b.reshape(NCORES, 128, S16)

    return dict(
        d_all=d_all, d_parts=d_parts, d_b=d_b, pieces=pieces, S=S, S16=S16,
        ROWS=ROWS, seg_key=seg_key, strm_o=strm_o, rowpos_o=rowpos_o,
    )


def _install_trace_shim(bass_utils):
    """Wire the NTFF profile hook that this image's antenv lacks, and make
    artifact upload local-only."""
    import types
    import contextlib
    import ctypes

    if "antenv.axon_hooks" not in sys.modules:
        mod = types.ModuleType("antenv.axon_hooks")
        mod._hook = None
        def set_axon_ntff_profile_hook(h):
            mod._hook = h
        def get_axon_ntff_profile_hook():
            return mod._hook
        mod.set_axon_ntff_profile_hook = set_axon_ntff_profile_hook
        mod.get_axon_ntff_profile_hook = get_axon_ntff_profile_hook
        sys.modules["antenv.axon_hooks"] = mod
        import antenv
        antenv.axon_hooks = mod

        so_path = "/opt/axon/libaxon_pjrt.so"
        if os.path.exists(so_path):
            lib = ctypes.CDLL(so_path)
            if hasattr(lib, "axon_start_nrt_profile"):
                lib.axon_start_nrt_profile.argtypes = [
                    ctypes.POINTER(ctypes.c_int64), ctypes.c_size_t]
                lib.axon_start_nrt_profile.restype = ctypes.c_int64
                lib.axon_stop_nrt_profile.argtypes = [ctypes.c_char_p]
                lib.axon_stop_nrt_profile.restype = ctypes.c_int64

                @contextlib.contextmanager
                def _hook(output_dir, device_ids):
                    import jax
                    jax.devices()
                    if device_ids:
                        ids = (ctypes.c_int64 * len(device_ids))(*device_ids)
                        rc = lib.axon_start_nrt_profile(ids, len(device_ids))
                    else:
                        rc = lib.axon_start_nrt_profile(None, 0)
                    if rc != 0:
                        raise RuntimeError(f"axon_start_nrt_profile rc={rc}")
                    try:
                        yield
                    finally:
                        n = lib.axon_stop_nrt_profile(str(output_dir).encode())
                        print(f"profile: {n} ntff file(s) -> {output_dir}",
                              file=sys.stderr)

                set_axon_ntff_profile_hook(_hook)

    bass_utils.upload_artifacts = lambda tmpdir: f"local://{tmpdir}"


_NC_CACHE = {}


def _build_nc(S, S16, ROWS, pieces, probe=False):
    import concourse.bacc as bacc
    import concourse.tile as tile
    from concourse import mybir
    from contextlib import ExitStack

    cache_key = (S, S16, ROWS, tuple(pieces), probe)
    if cache_key in _NC_CACHE:
        return _NC_CACHE[cache_key]

    f32 = mybir.dt.float32
    AF = mybir.ActivationFunctionType
    ALU = mybir.AluOpType

    bf = mybir.dt.bfloat16
    nc = bacc.Bacc("TRN2", target_bir_lowering=False, debug=False,
                   num_devices=NCORES)
    d_c_t = nc.dram_tensor("d_c", (NGROUP * 3, S), bf, kind="ExternalInput")
    d_b_t = nc.dram_tensor("d_b", (128, S16), f32, kind="ExternalInput")
    vec_t = nc.dram_tensor("vecs", (128, 5), f32, kind="ExternalInput")
    out_t = nc.dram_tensor("out", (NGROUP, 16, ROWS), f32, kind="ExternalOutput")

    import ml_dtypes
    nbf = ml_dtypes.bfloat16
    ind_d = np.zeros((NGROUP * 3, 128), nbf)
    ind_d[np.arange(128)[None, :] // 16 * 3 + np.arange(3)[:, None],
          np.arange(128)[None, :]] = 1.0
    ind_d_t = nc.inline_tensor(ind_d, "ind_d")
    ind_w = np.zeros((16, 128, 128), nbf)
    for j in range(16):
        ind_w[j, (np.arange(128) // 16) * 16 + j, np.arange(128)] = 1.0
    ind_w_t = nc.inline_tensor(
        np.ascontiguousarray(ind_w.transpose(1, 0, 2)).reshape(128, 16 * 128),
        "ind_w")

    with tile.TileContext(nc) as tc, ExitStack() as ctx:
        cpool = ctx.enter_context(tc.tile_pool(name="consts", bufs=1))
        lhsT_d = cpool.tile([NGROUP * 3, 128], bf)
        nc.sync.dma_start(lhsT_d[:], ind_d_t.ap())
        lhsT_w = cpool.tile([128, 16 * 128], bf)
        nc.sync.dma_start(lhsT_w[:], ind_w_t.ap())
        vec = cpool.tile([128, 5], f32)
        nc.sync.dma_start(vec[:], vec_t.ap())
        dbt = cpool.tile([128, S16], f32)
        nc.sync.dma_start(dbt[:], d_b_t.ap())
        winb = cpool.tile([128, S16], f32)
        # win_sin = sin(pi/c * d + pi/2); win = 0.5*(1+win_sin)
        nc.scalar.activation(winb[:], dbt[:], AF.Sin,
                             bias=vec[:, 3:4], scale=vec[:, 2:3])
        wh = cpool.tile([128, S16], bf)
        nc.scalar.copy(wh[:], winb[:])
        wl = cpool.tile([128, S16], bf)
        nc.vector.tensor_sub(wl[:], winb[:], wh[:])
        R = cpool.tile([128, ROWS], f32)

        dcp = ctx.enter_context(tc.tile_pool(name="dc", bufs=3))
        pdp = ctx.enter_context(tc.tile_pool(name="pd", bufs=2, space="PSUM"))
        pwp = ctx.enter_context(tc.tile_pool(name="pw", bufs=2, space="PSUM"))
        sqp = ctx.enter_context(tc.tile_pool(name="sq", bufs=3))
        g2p = ctx.enter_context(tc.tile_pool(name="g2", bufs=3))
        hep = ctx.enter_context(tc.tile_pool(name="he", bufs=3))

        for p, (o, c, m, ro) in enumerate(pieces):
            psz = c * m
            j, s0 = p % 16, (p // 16) * PIECE
            dc = dcp.tile([NGROUP * 3, PIECE], bf, tag="dc")
            nc.sync.dma_start(dc[:, :psz], d_c_t.ap()[:, o : o + psz])
            pd = pdp.tile([128, PIECE], f32, tag="pd")
            for h0 in range(0, psz, 512):
                h1 = min(h0 + 512, psz)
                nc.tensor.matmul(pd[:, h0:h1], lhsT_d[:], dc[:, h0:h1],
                                 start=True, stop=True)
            sq = sqp.tile([128, PIECE], f32, tag="sq")
            nc.scalar.activation(sq[:, :psz], pd[:, :psz], AF.Square,
                                 bias=vec[:, 0:1], scale=1.0)
            g2 = g2p.tile([128, PIECE], f32, tag="g2")
            nc.scalar.activation(g2[:, :psz], sq[:, :psz], AF.Exp,
                                 bias=vec[:, 4:5], scale=vec[:, 1:2])
            pw = pwp.tile([128, PIECE], f32, tag="pw")
            for h0 in range(0, psz, 512):
                h1 = min(h0 + 512, psz)
                nc.tensor.matmul(pw[:, h0:h1], lhsT_w[:, j * 128 : (j + 1) * 128],
                                 wh[:, s0 + h0 : s0 + h1], start=True, stop=False)
                nc.tensor.matmul(pw[:, h0:h1], lhsT_w[:, j * 128 : (j + 1) * 128],
                                 wl[:, s0 + h0 : s0 + h1], start=False, stop=True)
            he = hep.tile([128, PIECE], f32, tag="he")
            nc.vector.scalar_tensor_tensor(he[:, :psz], pw[:, :psz], 1.0,
                                           g2[:, :psz], op0=ALU.add,
                                           op1=ALU.mult)
            red_in = he[:, :psz].rearrange("q (c m) -> q c m", m=m)
            nc.vector.tensor_reduce(R[:, ro : ro + c], red_in,
                                    axis=mybir.AxisListType.X, op=ALU.add)

        for g in range(NGROUP):
            nc.sync.dma_start(out_t.ap()[g], R[g * 16 : (g + 1) * 16, :])

        if probe:
            import concourse.bass as bass  # noqa
            ppool = ctx.enter_context(tc.tile_pool(name="probe", bufs=1))
            pdat = ppool.tile([128, 1024], f32)
            pidx = ppool.tile([128, 128], mybir.dt.int16)
            pout = ppool.tile([128, 2048], f32)
            nc.vector.memset(pdat[:], 1.0)
            nc.vector.memset(pidx[:].bitcast(f32), 0.0)
            with nc.named_scope("probe_apgather_512"):
                nc.gpsimd.ap_gather(pout[:, :512].rearrange("p (i d) -> p i d", d=1),
                                    pdat[:].rearrange("p (e d) -> p e d", d=1),
                                    pidx[:, :32], channels=128,
                                    num_elems=1024, d=1, num_idxs=512)
            with nc.named_scope("probe_apgather_2048"):
                nc.gpsimd.ap_gather(pout[:, :2048].rearrange("p (i d) -> p i d", d=1),
                                    pdat[:].rearrange("p (e d) -> p e d", d=1),
                                    pidx[:, :128], channels=128,
                                    num_elems=1024, d=1, num_idxs=2048)
            with nc.named_scope("probe_apgather_d16"):
                nc.gpsimd.ap_gather(pout[:, :2048].rearrange("p (i d) -> p i d", d=16),
                                    pdat[:].rearrange("p (e d) -> p e d", d=16),
                                    pidx[:, :8], channels=128,
                                    num_elems=64, d=16, num_idxs=128)

    nc.compile()
    _NC_CACHE[cache_key] = nc
    return nc


def kernel(**inputs):
    global PAD_D
    feat = np.asarray(inputs["feat"], np.float32)
    distances = np.asarray(inputs["distances"], np.float32)
    src = np.asarray(inputs["src"])
    dst = np.asarray(inputs["dst"])
    cutoffs = np.asarray(inputs["interaction_cutoffs"], np.float32)
    mu = np.asarray(inputs["rbf_kernel_means"], np.float32)
    scal = np.asarray(inputs["rbf_kernel_scaling"], np.float32)
    ftu = np.asarray(inputs["features_to_use"], np.float32)

    assert np.all(cutoffs == cutoffs[0]), "per-k cutoffs unsupported"
    cutoff = float(cutoffs[0])
    PAD_D = cutoff  # win(cutoff) == 0 kills padding contributions

    lay = _host_layout(feat, distances, src, dst, ftu)
    S, S16, ROWS, pieces = lay["S"], lay["S16"], lay["ROWS"], lay["pieces"]

    kk = np.arange(128) % 16
    vecs = np.stack([
        -mu[kk],                                    # Square bias
        -scal[kk],                                  # Exp scale
        np.full(128, -math.pi / cutoff, np.float32),  # Sin scale (cos(x)=sin(pi/2-x))
        np.full(128, math.pi / 2, np.float32),       # Sin bias
        np.full(128, math.log(0.5), np.float32),     # Exp bias
    ], axis=1).astype(np.float32)

    probe = bool(int(os.environ.get("KERNEL_PROBE", "0")))
    trace = bool(int(os.environ.get("KERNEL_TRACE", "0")))
    nc = _build_nc(S, S16, ROWS, pieces, probe=probe)

    from concourse import bass_utils
    if trace:
        _install_trace_shim(bass_utils)
    in_maps = [
        {"d_c": np.ascontiguousarray(lay["d_parts"][c]),
         "d_b": np.ascontiguousarray(lay["d_b"][c]),
         "vecs": vecs}
        for c in range(NCORES)
    ]
    res = bass_utils.run_bass_kernel_spmd(
        nc, in_maps, core_ids=list(range(NCORES)), trace=trace,
        trace_cores=list(range(NCORES)) if trace else None,
    )
    LAST_RESULTS["res"] = res

    # gather/unshard: dev[core][g][k][row] -> out[v, t*K+k]
    dev = np.stack([r["out"] for r in res.results])  # (8, NGROUP, 16, ROWS)
    rows_all = dev.transpose(0, 1, 3, 2).reshape(NSTREAM, ROWS, K)
    seg_rows = rows_all[lay["strm_o"], lay["rowpos_o"]]  # (nseg, K)
    out = np.zeros((V * T, K), np.float32)
    seg_key = lay["seg_key"]
    if len(np.unique(seg_key)) == len(seg_key):
        out[seg_key] = seg_rows
    else:
        np.add.at(out, seg_key, seg_rows)
    return out.reshape(V, T * K).astype(np.float32)


if __name__ == "__main__":
    # smoke test with tiny random data through the same code paths
    rng = np.random.default_rng(0)
    nE, nV = 5000, 300
    feat = rng.integers(0, T, (nV, 1)).astype(np.float32)
    inputs = dict(
        feat=feat,
        distances=(rng.random((nE, 1)) * 12.0).astype(np.float32),
        src=rng.integers(0, nV, nE).astype(np.int32),
        dst=rng.integers(0, nV, nE).astype(np.int32),
        interaction_cutoffs=np.full(K, 12.0, np.float32),
        rbf_kernel_means=np.linspace(0, 12, K).astype(np.float32),
        rbf_kernel_scaling=np.ones(K, np.float32),
        features_to_use=np.arange(T, dtype=np.float32),
    )
    print(kernel(**inputs).sum())


# revision 10
# speedup vs baseline: 3.0412x; 1.2706x over previous
"""AtomicConv (gnn_message_passing) Trainium2 kernel.

out[v, t*K+k] = sum_{e: dst[e]=v, feat[src[e]]=t} exp(-scal_k*(d_e-mu_k)^2) * win(d_e)
with win(d) = 0.5*(cos(pi*d/cutoff)+1) for d <= cutoff.

Strategy (8 NeuronCores, edge segments dealt across cores):
  * Host: sort edges by (dst, src_type) -> contiguous (v,t) segments; deal
    segments round-robin by length over 64 streams (8 cores x 8 gpsimd-group
    streams).  Within a stream, segments of equal length m are adjacent, so a
    segment sum is a fixed-stride tensor_reduce([128, c, m]) - no scatter,
    gather or scan on device.
  * Device layout: partition p = (group g = p//16, filter k = p%16).  The
    per-edge distance stream of group g is broadcast to its 16 partitions with
    a 0/1 indicator matmul on the (otherwise idle) tensor engine -> PSUM.
  * ScalarE computes Square(d - mu_k) (per-partition bias) then
    Exp(-scal_k * sq + ln(0.5)); the cosine window 0.5*(1+sin(pi*d/c + pi/2))
    is computed once per slot in a 16x-smaller "blocked" layout and broadcast
    through the tensor engine too; VectorE fuses he = (win_sin + 1) * gauss
    and does the bucketed reduces.
  * Host unpermutes the dense per-stream row blocks into the (V, T*K) output.

The kernel is self-contained: shapes/sharding hardcoded for the
V=100000, E=3200000, K=16, T=4 problem (but layout is data-derived at call
time, so any same-shape input works).
"""

import math
import os
import sys

import numpy as np

sys.path.insert(0, "/opt/trn_rl_repo")

V, E, K, T = 100000, 3200000, 16, 4
NCORES = 8
NGROUP = 8  # streams per core == gpsimd groups
NSTREAM = NCORES * NGROUP
MAXSEG = 64  # segments longer than this are split into chunks
PIECE = 1024  # moving free dim (bf16 matmul max; 2 PSUM banks fp32)

PAD_D = None  # set to cutoff at runtime (win(cutoff) == 0)

LAST_RESULTS = {}  # test harness introspection


def _host_layout(feat, distances, src, dst, ftu):
    """Sort + deal edges; build device input arrays and unpermute metadata."""
    feat = np.asarray(feat, np.float32).reshape(-1)
    d = np.asarray(distances, np.float32).reshape(-1)
    src = np.asarray(src, np.int64).reshape(-1)
    dst = np.asarray(dst, np.int64).reshape(-1)
    ftu = np.asarray(ftu, np.float32).reshape(-1)
    nE = d.shape[0]
    assert ftu.shape[0] == T

    # src type index by value match against features_to_use (general one-hot)
    fs = feat[src]
    match = fs[:, None] == ftu[None, :]
    t_src = np.argmax(match, axis=1).astype(np.int64)
    valid = match.any(axis=1)

    key = dst * T + t_src
    if not valid.all():
        key = key[valid]
        d = d[valid]
    order = np.argsort(key, kind="stable")
    d_s = d[order]
    key_s = key[order]

    uk, uidx, ucnt = np.unique(key_s, return_index=True, return_counts=True)
    if ucnt.max(initial=0) > MAXSEG:
        nch = -(-ucnt // MAXSEG)
        seg_key = np.repeat(uk, nch)
        seg_len = np.full(int(nch.sum()), MAXSEG, np.int64)
        # trailing chunk lengths
        ends = np.cumsum(nch) - 1
        seg_len[ends] = ucnt - (nch - 1) * MAXSEG
        seg_start = np.concatenate([[0], np.cumsum(seg_len)[:-1]])
    else:
        seg_key, seg_start, seg_len = uk, uidx, ucnt.astype(np.int64)
    nseg = len(seg_key)

    # deal segments round-robin by length
    sorder = np.argsort(seg_len, kind="stable")
    slen_sorted = seg_len[sorder]
    lens, lcnt = np.unique(slen_sorted, return_counts=True)
    caps = -(-lcnt // NSTREAM)  # per-stream per-bucket segment capacity
    slot_off = np.concatenate([[0], np.cumsum(caps * lens)]).astype(np.int64)
    row_off = np.concatenate([[0], np.cumsum(caps)]).astype(np.int64)
    S_need = int(slot_off[-1])
    ROWS = int(row_off[-1])

    bstart = np.concatenate([[0], np.cumsum(lcnt)])
    rank = np.arange(nseg) - np.repeat(bstart[:-1], lcnt)
    b_of = np.repeat(np.arange(len(lens)), lcnt)
    strm = rank % NSTREAM
    sidx = rank // NSTREAM
    slotbase = slot_off[b_of] + sidx * lens[b_of]
    rowpos = row_off[b_of] + sidx
    inv = np.empty(nseg, np.int64)
    inv[sorder] = np.arange(nseg)
    strm_o = strm[inv]
    slotbase_o = slotbase[inv]
    rowpos_o = rowpos[inv]

    # per-edge slot placement
    e_seg = np.repeat(np.arange(nseg), seg_len)
    e_off = np.arange(len(d_s)) - np.repeat(seg_start, seg_len)
    e_strm = strm_o[e_seg]
    e_slot = slotbase_o[e_seg] + e_off

    S = -(-S_need // 16) * 16
    pad_d = float(PAD_D)
    d_all = np.full((NSTREAM, S), pad_d, np.float32)
    d_all[e_strm, e_slot] = d_s
    # 3-way bf16 split: d == dh + dm + dl to ~1e-7 abs
    import ml_dtypes
    bf16 = ml_dtypes.bfloat16
    dh = d_all.astype(bf16)
    r1 = d_all - dh.astype(np.float32)
    dm = r1.astype(bf16)
    r2 = r1 - dm.astype(np.float32)
    dl = r2.astype(bf16)
    # (NCORES, NGROUP, 3, S) -> rows g*3+part
    d_parts = np.stack([dh, dm, dl], axis=1).reshape(NSTREAM, 3, S)
    d_parts = d_parts.reshape(NCORES, NGROUP * 3, S)

    # piece list: (slot offset, segments, m, row offset)
    pieces = []
    for b in range(len(lens)):
        m = int(lens[b])
        cap = int(caps[b])
        o = int(slot_off[b])
        ro = int(row_off[b])
        left = cap
        while left > 0:
            c = min(PIECE // m, left)
            pieces.append((o, c, m, ro))
            o += c * m
            ro += c
            left -= c
    npieces = len(pieces)
    S16 = -(-npieces // 16) * PIECE

    # blocked layout (piece p -> partition j = p%16, slot16 = (p//16)*PIECE)
    d_all3 = d_all.reshape(NCORES, NGROUP, S)
    d_b = np.full((NCORES, NGROUP, 16, S16), pad_d, np.float32)
    for p, (o, c, m, ro) in enumerate(pieces):
        j, s0 = p % 16, (p // 16) * PIECE
        psz = c * m
        d_b[:, :, j, s0 : s0 + psz] = d_all3[:, :, o : o + psz]
    d_b = d_b.reshape(NCORES, 128, S16)

    return dict(
        d_all=d_all, d_parts=d_parts, d_b=d_b, pieces=pieces, S=S, S16=S16,
        ROWS=ROWS, seg_key=seg_key, strm_o=strm_o, rowpos_o=rowpos_o,
    )


def _install_trace_shim(bass_utils):
    """Wire the NTFF profile hook that this image's antenv lacks, and make
    artifact upload local-only."""
    import types
    import contextlib
    import ctypes

    if "antenv.axon_hooks" not in sys.modules:
        mod = types.ModuleType("antenv.axon_hooks")
        mod._hook = None
        def set_axon_ntff_profile_hook(h):
            mod._hook = h
        def get_axon_ntff_profile_hook():
            return mod._hook
        mod.set_axon_ntff_profile_hook = set_axon_ntff_profile_hook
        mod.get_axon_ntff_profile_hook = get_axon_ntff_profile_hook
        sys.modules["antenv.axon_hooks"] = mod
        import antenv
        antenv.axon_hooks = mod

        so_path = "/opt/axon/libaxon_pjrt.so"
        if os.path.exists(so_path):
            lib = ctypes.CDLL(so_path)
            if hasattr(lib, "axon_start_nrt_profile"):
                lib.axon_start_nrt_profile.argtypes = [
                    ctypes.POINTER(ctypes.c_int64), ctypes.c_size_t]
                lib.axon_start_nrt_profile.restype = ctypes.c_int64
                lib.axon_stop_nrt_profile.argtypes = [ctypes.c_char_p]
                lib.axon_stop_nrt_profile.restype = ctypes.c_int64

                @contextlib.contextmanager
                def _hook(output_dir, device_ids):
                    import jax
                    jax.devices()
                    if device_ids:
                        ids = (ctypes.c_int64 * len(device_ids))(*device_ids)
                        rc = lib.axon_start_nrt_profile(ids, len(device_ids))
                    else:
                        rc = lib.axon_start_nrt_profile(None, 0)
                    if rc != 0:
                        raise RuntimeError(f"axon_start_nrt_profile rc={rc}")
                    try:
                        yield
                    finally:
                        n = lib.axon_stop_nrt_profile(str(output_dir).encode())
                        print(f"profile: {n} ntff file(s) -> {output_dir}",
                              file=sys.stderr)

                set_axon_ntff_profile_hook(_hook)

    bass_utils.upload_artifacts = lambda tmpdir: f"local://{tmpdir}"


_NC_CACHE = {}


def _build_nc(S, S16, ROWS, pieces, probe=False):
    import concourse.bacc as bacc
    import concourse.tile as tile
    from concourse import mybir
    from contextlib import ExitStack

    cache_key = (S, S16, ROWS, tuple(pieces), probe)
    if cache_key in _NC_CACHE:
        return _NC_CACHE[cache_key]

    f32 = mybir.dt.float32
    AF = mybir.ActivationFunctionType
    ALU = mybir.AluOpType

    bf = mybir.dt.bfloat16
    nc = bacc.Bacc("TRN2", target_bir_lowering=False, debug=False,
                   num_devices=NCORES)
    d_c_t = nc.dram_tensor("d_c", (NGROUP * 3, S), bf, kind="ExternalInput")
    d_b_t = nc.dram_tensor("d_b", (128, S16), f32, kind="ExternalInput")
    vec_t = nc.dram_tensor("vecs", (128, 5), f32, kind="ExternalInput")
    out_t = nc.dram_tensor("out", (NGROUP, 16, ROWS), f32, kind="ExternalOutput")

    import ml_dtypes
    nbf = ml_dtypes.bfloat16
    ind_d = np.zeros((NGROUP * 3, 128), nbf)
    ind_d[np.arange(128)[None, :] // 16 * 3 + np.arange(3)[:, None],
          np.arange(128)[None, :]] = 1.0
    ind_d_t = nc.inline_tensor(ind_d, "ind_d")
    ind_w = np.zeros((16, 128, 128), nbf)
    for j in range(16):
        ind_w[j, (np.arange(128) // 16) * 16 + j, np.arange(128)] = 1.0
    ind_w_t = nc.inline_tensor(
        np.ascontiguousarray(ind_w.transpose(1, 0, 2)).reshape(128, 16 * 128),
        "ind_w")

    with tile.TileContext(nc) as tc, ExitStack() as ctx:
        cpool = ctx.enter_context(tc.tile_pool(name="consts", bufs=1))
        lhsT_d = cpool.tile([NGROUP * 3, 128], bf)
        nc.sync.dma_start(lhsT_d[:], ind_d_t.ap())
        lhsT_w = cpool.tile([128, 16 * 128], bf)
        nc.sync.dma_start(lhsT_w[:], ind_w_t.ap())
        vec = cpool.tile([128, 5], f32)
        nc.sync.dma_start(vec[:], vec_t.ap())
        dbt = cpool.tile([128, S16], f32)
        nc.sync.dma_start(dbt[:], d_b_t.ap())
        winb = cpool.tile([128, S16], f32)
        # win_sin = sin(pi/c * d + pi/2); win = 0.5*(1+win_sin)
        nc.scalar.activation(winb[:], dbt[:], AF.Sin,
                             bias=vec[:, 3:4], scale=vec[:, 2:3])
        wh = cpool.tile([128, S16], bf)
        nc.scalar.copy(wh[:], winb[:])
        R = cpool.tile([128, ROWS], f32)

        dcp = ctx.enter_context(tc.tile_pool(name="dc", bufs=3))
        pdp = ctx.enter_context(tc.tile_pool(name="pd", bufs=2, space="PSUM"))
        pwp = ctx.enter_context(tc.tile_pool(name="pw", bufs=2, space="PSUM"))
        sqp = ctx.enter_context(tc.tile_pool(name="sq", bufs=3))
        g2p = ctx.enter_context(tc.tile_pool(name="g2", bufs=3))
        hep = ctx.enter_context(tc.tile_pool(name="he", bufs=3))

        for p, (o, c, m, ro) in enumerate(pieces):
            psz = c * m
            j, s0 = p % 16, (p // 16) * PIECE
            dc = dcp.tile([NGROUP * 3, PIECE], bf, tag="dc")
            nc.sync.dma_start(dc[:, :psz], d_c_t.ap()[:, o : o + psz])
            pd = pdp.tile([128, PIECE], f32, tag="pd")
            for h0 in range(0, psz, 512):
                h1 = min(h0 + 512, psz)
                nc.tensor.matmul(pd[:, h0:h1], lhsT_d[:], dc[:, h0:h1],
                                 start=True, stop=True)
            sq = sqp.tile([128, PIECE], f32, tag="sq")
            nc.scalar.activation(sq[:, :psz], pd[:, :psz], AF.Square,
                                 bias=vec[:, 0:1], scale=1.0)
            g2 = g2p.tile([128, PIECE], f32, tag="g2")
            nc.scalar.activation(g2[:, :psz], sq[:, :psz], AF.Exp,
                                 bias=vec[:, 4:5], scale=vec[:, 1:2])
            pw = pwp.tile([128, PIECE], f32, tag="pw")
            for h0 in range(0, psz, 512):
                h1 = min(h0 + 512, psz)
                nc.tensor.matmul(pw[:, h0:h1], lhsT_w[:, j * 128 : (j + 1) * 128],
                                 wh[:, s0 + h0 : s0 + h1], start=True, stop=True)
            he = hep.tile([128, PIECE], bf, tag="he")
            nc.vector.scalar_tensor_tensor(he[:, :psz], pw[:, :psz], 1.0,
                                           g2[:, :psz], op0=ALU.add,
                                           op1=ALU.mult)
            red_in = he[:, :psz].rearrange("q (c m) -> q c m", m=m)
            nc.vector.tensor_reduce(R[:, ro : ro + c], red_in,
                                    axis=mybir.AxisListType.X, op=ALU.add)

        for g in range(NGROUP):
            nc.sync.dma_start(out_t.ap()[g], R[g * 16 : (g + 1) * 16, :])

        if probe:
            import concourse.bass as bass  # noqa
            ppool = ctx.enter_context(tc.tile_pool(name="probe", bufs=1))
            pdat = ppool.tile([128, 1024], f32)
            pidx = ppool.tile([128, 128], mybir.dt.int16)
            pout = ppool.tile([128, 2048], f32)
            nc.vector.memset(pdat[:], 1.0)
            nc.vector.memset(pidx[:].bitcast(f32), 0.0)
            with nc.named_scope("probe_apgather_512"):
                nc.gpsimd.ap_gather(pout[:, :512].rearrange("p (i d) -> p i d", d=1),
                                    pdat[:].rearrange("p (e d) -> p e d", d=1),
                                    pidx[:, :32], channels=128,
                                    num_elems=1024, d=1, num_idxs=512)
            with nc.named_scope("probe_apgather_2048"):
                nc.gpsimd.ap_gather(pout[:, :2048].rearrange("p (i d) -> p i d", d=1),
                                    pdat[:].rearrange("p (e d) -> p e d", d=1),
                                    pidx[:, :128], channels=128,
                                    num_elems=1024, d=1, num_idxs=2048)
            with nc.named_scope("probe_apgather_d16"):
                nc.gpsimd.ap_gather(pout[:, :2048].rearrange("p (i d) -> p i d", d=16),
                                    pdat[:].rearrange("p (e d) -> p e d", d=16),
                                    pidx[:, :8], channels=128,
                                    num_elems=64, d=16, num_idxs=128)

    nc.compile()
    _NC_CACHE[cache_key] = nc
    return nc


def kernel(**inputs):
    global PAD_D
    feat = np.asarray(inputs["feat"], np.float32)
    distances = np.asarray(inputs["distances"], np.float32)
    src = np.asarray(inputs["src"])
    dst = np.asarray(inputs["dst"])
    cutoffs = np.asarray(inputs["interaction_cutoffs"], np.float32)
    mu = np.asarray(inputs["rbf_kernel_means"], np.float32)
    scal = np.asarray(inputs["rbf_kernel_scaling"], np.float32)
    ftu = np.asarray(inputs["features_to_use"], np.float32)

    assert np.all(cutoffs == cutoffs[0]), "per-k cutoffs unsupported"
    cutoff = float(cutoffs[0])
    PAD_D = cutoff  # win(cutoff) == 0 kills padding contributions

    lay = _host_layout(feat, distances, src, dst, ftu)
    S, S16, ROWS, pieces = lay["S"], lay["S16"], lay["ROWS"], lay["pieces"]

    kk = np.arange(128) % 16
    vecs = np.stack([
        -mu[kk],                                    # Square bias
        -scal[kk],                                  # Exp scale
        np.full(128, -math.pi / cutoff, np.float32),  # Sin scale (cos(x)=sin(pi/2-x))
        np.full(128, math.pi / 2, np.float32),       # Sin bias
        np.full(128, math.log(0.5), np.float32),     # Exp bias
    ], axis=1).astype(np.float32)

    probe = bool(int(os.environ.get("KERNEL_PROBE", "0")))
    trace = bool(int(os.environ.get("KERNEL_TRACE", "0")))
    nc = _build_nc(S, S16, ROWS, pieces, probe=probe)

    from concourse import bass_utils
    if trace:
        _install_trace_shim(bass_utils)
    in_maps = [
        {"d_c": np.ascontiguousarray(lay["d_parts"][c]),
         "d_b": np.ascontiguousarray(lay["d_b"][c]),
         "vecs": vecs}
        for c in range(NCORES)
    ]
    res = bass_utils.run_bass_kernel_spmd(
        nc, in_maps, core_ids=list(range(NCORES)), trace=trace,
        trace_cores=list(range(NCORES)) if trace else None,
    )
    LAST_RESULTS["res"] = res

    # gather/unshard: dev[core][g][k][row] -> out[v, t*K+k]
    dev = np.stack([r["out"] for r in res.results])  # (8, NGROUP, 16, ROWS)
    rows_all = dev.transpose(0, 1, 3, 2).reshape(NSTREAM, ROWS, K)
    seg_rows = rows_all[lay["strm_o"], lay["rowpos_o"]]  # (nseg, K)
    out = np.zeros((V * T, K), np.float32)
    seg_key = lay["seg_key"]
    if len(np.unique(seg_key)) == len(seg_key):
        out[seg_key] = seg_rows
    else:
        np.add.at(out, seg_key, seg_rows)
    return out.reshape(V, T * K).astype(np.float32)


if __name__ == "__main__":
    # smoke test with tiny random data through the same code paths
    rng = np.random.default_rng(0)
    nE, nV = 5000, 300
    feat = rng.integers(0, T, (nV, 1)).astype(np.float32)
    inputs = dict(
        feat=feat,
        distances=(rng.random((nE, 1)) * 12.0).astype(np.float32),
        src=rng.integers(0, nV, nE).astype(np.int32),
        dst=rng.integers(0, nV, nE).astype(np.int32),
        interaction_cutoffs=np.full(K, 12.0, np.float32),
        rbf_kernel_means=np.linspace(0, 12, K).astype(np.float32),
        rbf_kernel_scaling=np.ones(K, np.float32),
        features_to_use=np.arange(T, dtype=np.float32),
    )
    print(kernel(**inputs).sum())


# revision 14
# speedup vs baseline: 3.5577x; 1.1698x over previous
"""AtomicConv (gnn_message_passing) Trainium2 kernel.

out[v, t*K+k] = sum_{e: dst[e]=v, feat[src[e]]=t} exp(-scal_k*(d_e-mu_k)^2) * win(d_e)
with win(d) = 0.5*(cos(pi*d/cutoff)+1) for d <= cutoff.

Strategy (8 NeuronCores, edge segments dealt across cores):
  * Host: sort edges by (dst, src_type) -> contiguous (v,t) segments; deal
    segments round-robin by length over 64 streams (8 cores x 8 gpsimd-group
    streams).  Within a stream, segments of equal length m are adjacent, so a
    segment sum is a fixed-stride tensor_reduce([128, c, m]) - no scatter,
    gather or scan on device.
  * Device layout: partition p = (group g = p//16, filter k = p%16).  The
    per-edge distance stream of group g is broadcast to its 16 partitions with
    a 0/1 indicator matmul on the (otherwise idle) tensor engine -> PSUM.
  * ScalarE computes Square(d - mu_k) (per-partition bias) then
    Exp(-scal_k * sq + ln(0.5)); the cosine window 0.5*(1+sin(pi*d/c + pi/2))
    is computed once per slot in a 16x-smaller "blocked" layout and broadcast
    through the tensor engine too; VectorE fuses he = (win_sin + 1) * gauss
    and does the bucketed reduces.
  * Host unpermutes the dense per-stream row blocks into the (V, T*K) output.

The kernel is self-contained: shapes/sharding hardcoded for the
V=100000, E=3200000, K=16, T=4 problem (but layout is data-derived at call
time, so any same-shape input works).
"""

import math
import os
import sys

import numpy as np

sys.path.insert(0, "/opt/trn_rl_repo")

V, E, K, T = 100000, 3200000, 16, 4
NCORES = 8
NGROUP = 8  # streams per core == gpsimd groups
NSTREAM = NCORES * NGROUP
MAXSEG = 64  # segments longer than this are split into chunks
PIECE = 1024  # moving free dim (bf16 matmul max; 2 PSUM banks fp32)

PAD_D = None  # set to cutoff at runtime (win(cutoff) == 0)

LAST_RESULTS = {}  # test harness introspection


def _host_layout(feat, distances, src, dst, ftu):
    """Sort + deal edges; build device input arrays and unpermute metadata."""
    feat = np.asarray(feat, np.float32).reshape(-1)
    d = np.asarray(distances, np.float32).reshape(-1)
    src = np.asarray(src, np.int64).reshape(-1)
    dst = np.asarray(dst, np.int64).reshape(-1)
    ftu = np.asarray(ftu, np.float32).reshape(-1)
    nE = d.shape[0]
    assert ftu.shape[0] == T

    # src type index by value match against features_to_use (general one-hot)
    fs = feat[src]
    match = fs[:, None] == ftu[None, :]
    t_src = np.argmax(match, axis=1).astype(np.int64)
    valid = match.any(axis=1)

    key = dst * T + t_src
    if not valid.all():
        key = key[valid]
        d = d[valid]
    order = np.argsort(key, kind="stable")
    d_s = d[order]
    key_s = key[order]

    uk, uidx, ucnt = np.unique(key_s, return_index=True, return_counts=True)
    if ucnt.max(initial=0) > MAXSEG:
        nch = -(-ucnt // MAXSEG)
        seg_key = np.repeat(uk, nch)
        seg_len = np.full(int(nch.sum()), MAXSEG, np.int64)
        # trailing chunk lengths
        ends = np.cumsum(nch) - 1
        seg_len[ends] = ucnt - (nch - 1) * MAXSEG
        seg_start = np.concatenate([[0], np.cumsum(seg_len)[:-1]])
    else:
        seg_key, seg_start, seg_len = uk, uidx, ucnt.astype(np.int64)
    nseg = len(seg_key)

    # deal segments round-robin by length
    sorder = np.argsort(seg_len, kind="stable")
    slen_sorted = seg_len[sorder]
    lens, lcnt = np.unique(slen_sorted, return_counts=True)
    caps = -(-lcnt // NSTREAM)  # per-stream per-bucket segment capacity
    slot_off = np.concatenate([[0], np.cumsum(caps * lens)]).astype(np.int64)
    row_off = np.concatenate([[0], np.cumsum(caps)]).astype(np.int64)
    S_need = int(slot_off[-1])
    ROWS = int(row_off[-1])

    bstart = np.concatenate([[0], np.cumsum(lcnt)])
    rank = np.arange(nseg) - np.repeat(bstart[:-1], lcnt)
    b_of = np.repeat(np.arange(len(lens)), lcnt)
    strm = rank % NSTREAM
    sidx = rank // NSTREAM
    slotbase = slot_off[b_of] + sidx * lens[b_of]
    rowpos = row_off[b_of] + sidx
    inv = np.empty(nseg, np.int64)
    inv[sorder] = np.arange(nseg)
    strm_o = strm[inv]
    slotbase_o = slotbase[inv]
    rowpos_o = rowpos[inv]

    # per-edge slot placement
    e_seg = np.repeat(np.arange(nseg), seg_len)
    e_off = np.arange(len(d_s)) - np.repeat(seg_start, seg_len)
    e_strm = strm_o[e_seg]
    e_slot = slotbase_o[e_seg] + e_off

    S = -(-S_need // 16) * 16
    pad_d = float(PAD_D)
    d_all = np.full((NSTREAM, S), pad_d, np.float32)
    d_all[e_strm, e_slot] = d_s
    # 3-way bf16 split: d == dh + dm + dl to ~1e-7 abs
    import ml_dtypes
    bf16 = ml_dtypes.bfloat16
    dh = d_all.astype(bf16)
    r1 = d_all - dh.astype(np.float32)
    dm = r1.astype(bf16)
    r2 = r1 - dm.astype(np.float32)
    dl = r2.astype(bf16)
    q_all = d_all.astype(np.float64)
    q_all = (q_all * q_all).astype(np.float32)
    qh = q_all.astype(bf16)
    s1 = q_all - qh.astype(np.float32)
    qm = s1.astype(bf16)
    s2 = s1 - qm.astype(np.float32)
    ql = s2.astype(bf16)
    # rows: ch*{dh,dm,dl}, cl*{dh,dm}, cl2*{dh}, scal*{qh,qm,ql}
    d_parts = np.stack([dh, dm, dl, dh, dm, dh, qh, qm, ql],
                       axis=1).reshape(NSTREAM, 9, S)
    d_parts = d_parts.reshape(NCORES, NGROUP * 9, S)

    # piece list: (slot offset, segments, m, row offset)
    pieces = []
    for b in range(len(lens)):
        m = int(lens[b])
        cap = int(caps[b])
        o = int(slot_off[b])
        ro = int(row_off[b])
        left = cap
        while left > 0:
            c = min(PIECE // m, left)
            pieces.append((o, c, m, ro))
            o += c * m
            ro += c
            left -= c
    npieces = len(pieces)
    S16 = -(-npieces // 16) * PIECE

    # blocked layout (piece p -> partition j = p%16, slot16 = (p//16)*PIECE)
    d_all3 = d_all.reshape(NCORES, NGROUP, S)
    d_b = np.full((NCORES, NGROUP, 16, S16), pad_d, np.float32)
    for p, (o, c, m, ro) in enumerate(pieces):
        j, s0 = p % 16, (p // 16) * PIECE
        psz = c * m
        d_b[:, :, j, s0 : s0 + psz] = d_all3[:, :, o : o + psz]
    d_b = d_b.reshape(NCORES, 128, S16)

    return dict(
        d_all=d_all, d_parts=d_parts, d_b=d_b, pieces=pieces, S=S, S16=S16,
        ROWS=ROWS, seg_key=seg_key, strm_o=strm_o, rowpos_o=rowpos_o,
    )


def _install_trace_shim(bass_utils):
    """Wire the NTFF profile hook that this image's antenv lacks, and make
    artifact upload local-only."""
    import types
    import contextlib
    import ctypes

    if "antenv.axon_hooks" not in sys.modules:
        mod = types.ModuleType("antenv.axon_hooks")
        mod._hook = None
        def set_axon_ntff_profile_hook(h):
            mod._hook = h
        def get_axon_ntff_profile_hook():
            return mod._hook
        mod.set_axon_ntff_profile_hook = set_axon_ntff_profile_hook
        mod.get_axon_ntff_profile_hook = get_axon_ntff_profile_hook
        sys.modules["antenv.axon_hooks"] = mod
        import antenv
        antenv.axon_hooks = mod

        so_path = "/opt/axon/libaxon_pjrt.so"
        if os.path.exists(so_path):
            lib = ctypes.CDLL(so_path)
            if hasattr(lib, "axon_start_nrt_profile"):
                lib.axon_start_nrt_profile.argtypes = [
                    ctypes.POINTER(ctypes.c_int64), ctypes.c_size_t]
                lib.axon_start_nrt_profile.restype = ctypes.c_int64
                lib.axon_stop_nrt_profile.argtypes = [ctypes.c_char_p]
                lib.axon_stop_nrt_profile.restype = ctypes.c_int64

                @contextlib.contextmanager
                def _hook(output_dir, device_ids):
                    import jax
                    jax.devices()
                    if device_ids:
                        ids = (ctypes.c_int64 * len(device_ids))(*device_ids)
                        rc = lib.axon_start_nrt_profile(ids, len(device_ids))
                    else:
                        rc = lib.axon_start_nrt_profile(None, 0)
                    if rc != 0:
                        raise RuntimeError(f"axon_start_nrt_profile rc={rc}")
                    try:
                        yield
                    finally:
                        n = lib.axon_stop_nrt_profile(str(output_dir).encode())
                        print(f"profile: {n} ntff file(s) -> {output_dir}",
                              file=sys.stderr)

                set_axon_ntff_profile_hook(_hook)

    bass_utils.upload_artifacts = lambda tmpdir: f"local://{tmpdir}"


_NC_CACHE = {}


def _build_nc(S, S16, ROWS, pieces, probe=False):
    import concourse.bacc as bacc
    import concourse.tile as tile
    from concourse import mybir
    from contextlib import ExitStack

    cache_key = (S, S16, ROWS, tuple(pieces), probe)
    if cache_key in _NC_CACHE:
        return _NC_CACHE[cache_key]

    f32 = mybir.dt.float32
    AF = mybir.ActivationFunctionType
    ALU = mybir.AluOpType

    bf = mybir.dt.bfloat16
    nc = bacc.Bacc("TRN2", target_bir_lowering=False, debug=False,
                   num_devices=NCORES)
    d_c_t = nc.dram_tensor("d_c", (NGROUP * 9, S), bf, kind="ExternalInput")
    d_b_t = nc.dram_tensor("d_b", (128, S16), f32, kind="ExternalInput")
    vec_t = nc.dram_tensor("vecs", (128, 5), f32, kind="ExternalInput")
    cof_t = nc.dram_tensor("cofs", (NGROUP * 9, 128), f32, kind="ExternalInput")
    out_t = nc.dram_tensor("out", (NGROUP, 16, ROWS), f32, kind="ExternalOutput")

    import ml_dtypes
    nbf = ml_dtypes.bfloat16
    ind_w = np.zeros((16, 128, 128), nbf)
    for j in range(16):
        ind_w[j, (np.arange(128) // 16) * 16 + j, np.arange(128)] = 1.0
    ind_w_t = nc.inline_tensor(
        np.ascontiguousarray(ind_w.transpose(1, 0, 2)).reshape(128, 16 * 128),
        "ind_w")

    with tile.TileContext(nc) as tc, ExitStack() as ctx:
        cpool = ctx.enter_context(tc.tile_pool(name="consts", bufs=1))
        cof = cpool.tile([NGROUP * 9, 128], f32)
        nc.sync.dma_start(cof[:], cof_t.ap())
        lhsT_d = cpool.tile([NGROUP * 9, 128], bf)
        nc.vector.tensor_copy(lhsT_d[:], cof[:])
        lhsT_w = cpool.tile([128, 16 * 128], bf)
        nc.sync.dma_start(lhsT_w[:], ind_w_t.ap())
        vec = cpool.tile([128, 5], f32)
        nc.sync.dma_start(vec[:], vec_t.ap())
        dbt = cpool.tile([128, S16], f32)
        nc.sync.dma_start(dbt[:], d_b_t.ap())
        winb = cpool.tile([128, S16], f32)
        # win_sin = sin(pi/c * d + pi/2); win = 0.5*(1+win_sin)
        nc.scalar.activation(winb[:], dbt[:], AF.Sin,
                             bias=vec[:, 3:4], scale=vec[:, 2:3])
        w05 = cpool.tile([128, S16], f32)
        nc.vector.tensor_scalar(w05[:], winb[:], 0.5, 0.5,
                                op0=ALU.mult, op1=ALU.add)
        w05c = cpool.tile([128, S16], f32)
        nc.vector.tensor_scalar(w05c[:], w05[:], 1e-13, None, op0=ALU.max)
        lnw = cpool.tile([128, S16], f32)
        nc.scalar.activation(lnw[:], w05c[:], AF.Ln)
        nlnw = cpool.tile([128, S16], f32)
        nc.vector.tensor_scalar(nlnw[:], lnw[:], -1.0, None, op0=ALU.mult)
        wh = cpool.tile([128, S16], bf)
        nc.vector.tensor_copy(wh[:], nlnw[:])
        R = cpool.tile([128, ROWS], f32)

        dcp = ctx.enter_context(tc.tile_pool(name="dc", bufs=3))
        pdp = ctx.enter_context(tc.tile_pool(name="pd", bufs=4, space="PSUM"))
        hep = ctx.enter_context(tc.tile_pool(name="he", bufs=3))

        for p, (o, c, m, ro) in enumerate(pieces):
            psz = c * m
            j, s0 = p % 16, (p // 16) * PIECE
            dc = dcp.tile([NGROUP * 9, PIECE], bf, tag="dc")
            nc.sync.dma_start(dc[:, :psz], d_c_t.ap()[:, o : o + psz])
            pd = pdp.tile([128, PIECE], f32, tag="pd")
            for h0 in range(0, psz, 512):
                h1 = min(h0 + 512, psz)
                nc.tensor.matmul(pd[:, h0:h1], lhsT_d[:], dc[:, h0:h1],
                                 start=True, stop=False)
                nc.tensor.matmul(pd[:, h0:h1], lhsT_w[:, j * 128 : (j + 1) * 128],
                                 wh[:, s0 + h0 : s0 + h1], start=False, stop=True)
            he = hep.tile([128, PIECE], bf, tag="he")
            nc.scalar.activation(he[:, :psz], pd[:, :psz], AF.Exp,
                                 bias=vec[:, 0:1], scale=vec[:, 1:2])
            red_in = he[:, :psz].rearrange("q (c m) -> q c m", m=m)
            nc.vector.tensor_reduce(R[:, ro : ro + c], red_in,
                                    axis=mybir.AxisListType.X, op=ALU.add)

        for g in range(NGROUP):
            nc.sync.dma_start(out_t.ap()[g], R[g * 16 : (g + 1) * 16, :])

        if probe:
            import concourse.bass as bass  # noqa
            ppool = ctx.enter_context(tc.tile_pool(name="probe", bufs=1))
            pdat = ppool.tile([128, 1024], f32)
            pidx = ppool.tile([128, 128], mybir.dt.int16)
            pout = ppool.tile([128, 2048], f32)
            nc.vector.memset(pdat[:], 1.0)
            nc.vector.memset(pidx[:].bitcast(f32), 0.0)
            with nc.named_scope("probe_apgather_512"):
                nc.gpsimd.ap_gather(pout[:, :512].rearrange("p (i d) -> p i d", d=1),
                                    pdat[:].rearrange("p (e d) -> p e d", d=1),
                                    pidx[:, :32], channels=128,
                                    num_elems=1024, d=1, num_idxs=512)
            with nc.named_scope("probe_apgather_2048"):
                nc.gpsimd.ap_gather(pout[:, :2048].rearrange("p (i d) -> p i d", d=1),
                                    pdat[:].rearrange("p (e d) -> p e d", d=1),
                                    pidx[:, :128], channels=128,
                                    num_elems=1024, d=1, num_idxs=2048)
            with nc.named_scope("probe_apgather_d16"):
                nc.gpsimd.ap_gather(pout[:, :2048].rearrange("p (i d) -> p i d", d=16),
                                    pdat[:].rearrange("p (e d) -> p e d", d=16),
                                    pidx[:, :8], channels=128,
                                    num_elems=64, d=16, num_idxs=128)

    nc.compile()
    _NC_CACHE[cache_key] = nc
    return nc


def kernel(**inputs):
    global PAD_D
    feat = np.asarray(inputs["feat"], np.float32)
    distances = np.asarray(inputs["distances"], np.float32)
    src = np.asarray(inputs["src"])
    dst = np.asarray(inputs["dst"])
    cutoffs = np.asarray(inputs["interaction_cutoffs"], np.float32)
    mu = np.asarray(inputs["rbf_kernel_means"], np.float32)
    scal = np.asarray(inputs["rbf_kernel_scaling"], np.float32)
    ftu = np.asarray(inputs["features_to_use"], np.float32)

    assert np.all(cutoffs == cutoffs[0]), "per-k cutoffs unsupported"
    cutoff = float(cutoffs[0])
    PAD_D = cutoff  # win(cutoff) == 0 kills padding contributions

    lay = _host_layout(feat, distances, src, dst, ftu)
    S, S16, ROWS, pieces = lay["S"], lay["S16"], lay["ROWS"], lay["pieces"]

    kk = np.arange(128) % 16
    # he = Exp(-(scal*q - 2*scal*mu*d + nlnw) - scal*mu^2) = gauss * win
    vecs = np.stack([
        (-scal[kk].astype(np.float64) * mu[kk].astype(np.float64) ** 2
         ).astype(np.float32),                       # Exp bias
        np.full(128, -1.0, np.float32),              # Exp scale
        np.full(128, -math.pi / cutoff, np.float32),  # Sin scale
        np.full(128, math.pi / 2, np.float32),       # Sin bias
        np.full(128, 0.0, np.float32),
    ], axis=1).astype(np.float32)
    import ml_dtypes
    nbf = ml_dtypes.bfloat16
    cd = (-2.0 * scal[kk].astype(np.float64) * mu[kk].astype(np.float64))
    ch = cd.astype(nbf).astype(np.float64)
    cl = (cd - ch).astype(nbf).astype(np.float64)
    cl2 = ((cd - ch) - cl).astype(nbf).astype(np.float32)
    sh = scal[kk].astype(nbf).astype(np.float32)
    cofs = np.zeros((NGROUP * 9, 128), np.float32)
    pp = np.arange(128)
    gg = pp // 16
    for r, coef in enumerate([ch, ch, ch, cl, cl, cl2, sh, sh, sh]):
        cofs[gg * 9 + r, pp] = coef.astype(np.float32)[pp]

    probe = bool(int(os.environ.get("KERNEL_PROBE", "0")))
    trace = bool(int(os.environ.get("KERNEL_TRACE", "0")))
    nc = _build_nc(S, S16, ROWS, pieces, probe=probe)

    from concourse import bass_utils
    if trace:
        _install_trace_shim(bass_utils)
    in_maps = [
        {"d_c": np.ascontiguousarray(lay["d_parts"][c]),
         "d_b": np.ascontiguousarray(lay["d_b"][c]),
         "vecs": vecs, "cofs": cofs}
        for c in range(NCORES)
    ]
    res = bass_utils.run_bass_kernel_spmd(
        nc, in_maps, core_ids=list(range(NCORES)), trace=trace,
        trace_cores=list(range(NCORES)) if trace else None,
    )
    LAST_RESULTS["res"] = res

    # gather/unshard: dev[core][g][k][row] -> out[v, t*K+k]
    dev = np.stack([r["out"] for r in res.results])  # (8, NGROUP, 16, ROWS)
    rows_all = dev.transpose(0, 1, 3, 2).reshape(NSTREAM, ROWS, K)
    seg_rows = rows_all[lay["strm_o"], lay["rowpos_o"]]  # (nseg, K)
    out = np.zeros((V * T, K), np.float32)
    seg_key = lay["seg_key"]
    if len(np.unique(seg_key)) == len(seg_key):
        out[seg_key] = seg_rows
    else:
        np.add.at(out, seg_key, seg_rows)
    return out.reshape(V, T * K).astype(np.float32)


if __name__ == "__main__":
    # smoke test with tiny random data through the same code paths
    rng = np.random.default_rng(0)
    nE, nV = 5000, 300
    feat = rng.integers(0, T, (nV, 1)).astype(np.float32)
    inputs = dict(
        feat=feat,
        distances=(rng.random((nE, 1)) * 12.0).astype(np.float32),
        src=rng.integers(0, nV, nE).astype(np.int32),
        dst=rng.integers(0, nV, nE).astype(np.int32),
        interaction_cutoffs=np.full(K, 12.0, np.float32),
        rbf_kernel_means=np.linspace(0, 12, K).astype(np.float32),
        rbf_kernel_scaling=np.ones(K, np.float32),
        features_to_use=np.arange(T, dtype=np.float32),
    )
    print(kernel(**inputs).sum())


# revision 15
# speedup vs baseline: 4.1507x; 1.1667x over previous
"""AtomicConv (gnn_message_passing) Trainium2 kernel.

out[v, t*K+k] = sum_{e: dst[e]=v, feat[src[e]]=t} exp(-scal_k*(d_e-mu_k)^2) * win(d_e)
with win(d) = 0.5*(cos(pi*d/cutoff)+1) for d <= cutoff.

Strategy (8 NeuronCores, edge segments dealt across cores):
  * Host: sort edges by (dst, src_type) -> contiguous (v,t) segments; deal
    segments round-robin by length over 64 streams (8 cores x 8 gpsimd-group
    streams).  Within a stream, segments of equal length m are adjacent, so a
    segment sum is a fixed-stride tensor_reduce([128, c, m]) - no scatter,
    gather or scan on device.
  * Device layout: partition p = (group g = p//16, filter k = p%16).  The
    per-edge distance stream of group g is broadcast to its 16 partitions with
    a 0/1 indicator matmul on the (otherwise idle) tensor engine -> PSUM.
  * ScalarE computes Square(d - mu_k) (per-partition bias) then
    Exp(-scal_k * sq + ln(0.5)); the cosine window 0.5*(1+sin(pi*d/c + pi/2))
    is computed once per slot in a 16x-smaller "blocked" layout and broadcast
    through the tensor engine too; VectorE fuses he = (win_sin + 1) * gauss
    and does the bucketed reduces.
  * Host unpermutes the dense per-stream row blocks into the (V, T*K) output.

The kernel is self-contained: shapes/sharding hardcoded for the
V=100000, E=3200000, K=16, T=4 problem (but layout is data-derived at call
time, so any same-shape input works).
"""

import math
import os
import sys

import numpy as np

sys.path.insert(0, "/opt/trn_rl_repo")

V, E, K, T = 100000, 3200000, 16, 4
NCORES = 8
NGROUP = 8  # streams per core == gpsimd groups
NSTREAM = NCORES * NGROUP
MAXSEG = 64  # segments longer than this are split into chunks
PIECE = 1024  # moving free dim (bf16 matmul max; 2 PSUM banks fp32)

PAD_D = None  # set to cutoff at runtime (win(cutoff) == 0)

LAST_RESULTS = {}  # test harness introspection


def _host_layout(feat, distances, src, dst, ftu):
    """Sort + deal edges; build device input arrays and unpermute metadata."""
    feat = np.asarray(feat, np.float32).reshape(-1)
    d = np.asarray(distances, np.float32).reshape(-1)
    src = np.asarray(src, np.int64).reshape(-1)
    dst = np.asarray(dst, np.int64).reshape(-1)
    ftu = np.asarray(ftu, np.float32).reshape(-1)
    nE = d.shape[0]
    assert ftu.shape[0] == T

    # src type index by value match against features_to_use (general one-hot)
    fs = feat[src]
    match = fs[:, None] == ftu[None, :]
    t_src = np.argmax(match, axis=1).astype(np.int64)
    valid = match.any(axis=1)

    key = dst * T + t_src
    if not valid.all():
        key = key[valid]
        d = d[valid]
    order = np.argsort(key, kind="stable")
    d_s = d[order]
    key_s = key[order]

    uk, uidx, ucnt = np.unique(key_s, return_index=True, return_counts=True)
    if ucnt.max(initial=0) > MAXSEG:
        nch = -(-ucnt // MAXSEG)
        seg_key = np.repeat(uk, nch)
        seg_len = np.full(int(nch.sum()), MAXSEG, np.int64)
        # trailing chunk lengths
        ends = np.cumsum(nch) - 1
        seg_len[ends] = ucnt - (nch - 1) * MAXSEG
        seg_start = np.concatenate([[0], np.cumsum(seg_len)[:-1]])
    else:
        seg_key, seg_start, seg_len = uk, uidx, ucnt.astype(np.int64)
    nseg = len(seg_key)

    # deal segments round-robin by length
    sorder = np.argsort(seg_len, kind="stable")
    slen_sorted = seg_len[sorder]
    lens, lcnt = np.unique(slen_sorted, return_counts=True)
    caps = -(-lcnt // NSTREAM)  # per-stream per-bucket segment capacity
    slot_off = np.concatenate([[0], np.cumsum(caps * lens)]).astype(np.int64)
    row_off = np.concatenate([[0], np.cumsum(caps)]).astype(np.int64)
    S_need = int(slot_off[-1])
    ROWS = int(row_off[-1])

    bstart = np.concatenate([[0], np.cumsum(lcnt)])
    rank = np.arange(nseg) - np.repeat(bstart[:-1], lcnt)
    b_of = np.repeat(np.arange(len(lens)), lcnt)
    strm = rank % NSTREAM
    sidx = rank // NSTREAM
    slotbase = slot_off[b_of] + sidx * lens[b_of]
    rowpos = row_off[b_of] + sidx
    inv = np.empty(nseg, np.int64)
    inv[sorder] = np.arange(nseg)
    strm_o = strm[inv]
    slotbase_o = slotbase[inv]
    rowpos_o = rowpos[inv]

    # per-edge slot placement
    e_seg = np.repeat(np.arange(nseg), seg_len)
    e_off = np.arange(len(d_s)) - np.repeat(seg_start, seg_len)
    e_strm = strm_o[e_seg]
    e_slot = slotbase_o[e_seg] + e_off

    S = -(-S_need // 16) * 16
    pad_d = float(PAD_D)
    d_all = np.full((NSTREAM, S), pad_d, np.float32)
    d_all[e_strm, e_slot] = d_s
    # 3-way bf16 split: d == dh + dm + dl to ~1e-7 abs
    import ml_dtypes
    bf16 = ml_dtypes.bfloat16
    dh = d_all.astype(bf16)
    r1 = d_all - dh.astype(np.float32)
    dm = r1.astype(bf16)
    r2 = r1 - dm.astype(np.float32)
    dl = r2.astype(bf16)
    q_all = d_all.astype(np.float64)
    q_all = (q_all * q_all).astype(np.float32)
    qh = q_all.astype(bf16)
    s1 = q_all - qh.astype(np.float32)
    qm = s1.astype(bf16)
    s2 = s1 - qm.astype(np.float32)
    ql = s2.astype(bf16)
    # rows: ch*{dh,dm,dl}, cl*{dh,dm}, cl2*{dh}, scal*{qh,qm,ql}
    d_parts = np.stack([dh, dm, dl, dh, dm, dh, qh, qm, ql],
                       axis=1).reshape(NSTREAM, 9, S)
    d_parts = d_parts.reshape(NCORES, NGROUP * 9, S)

    # piece list: (slot offset, segments, m, row offset)
    pieces = []
    for b in range(len(lens)):
        m = int(lens[b])
        cap = int(caps[b])
        o = int(slot_off[b])
        ro = int(row_off[b])
        left = cap
        while left > 0:
            c = min(PIECE // m, left)
            pieces.append((o, c, m, ro))
            o += c * m
            ro += c
            left -= c
    npieces = len(pieces)
    S16 = -(-npieces // 16) * PIECE

    # blocked layout (piece p -> partition j = p%16, slot16 = (p//16)*PIECE)
    d_all3 = d_all.reshape(NCORES, NGROUP, S)
    d_b = np.full((NCORES, NGROUP, 16, S16), pad_d, np.float32)
    for p, (o, c, m, ro) in enumerate(pieces):
        j, s0 = p % 16, (p // 16) * PIECE
        psz = c * m
        d_b[:, :, j, s0 : s0 + psz] = d_all3[:, :, o : o + psz]
    d_b = d_b.reshape(NCORES, 128, S16)

    return dict(
        d_all=d_all, d_parts=d_parts, d_b=d_b, pieces=pieces, S=S, S16=S16,
        ROWS=ROWS, seg_key=seg_key, strm_o=strm_o, rowpos_o=rowpos_o,
    )


def _install_trace_shim(bass_utils):
    """Wire the NTFF profile hook that this image's antenv lacks, and make
    artifact upload local-only."""
    import types
    import contextlib
    import ctypes

    if "antenv.axon_hooks" not in sys.modules:
        mod = types.ModuleType("antenv.axon_hooks")
        mod._hook = None
        def set_axon_ntff_profile_hook(h):
            mod._hook = h
        def get_axon_ntff_profile_hook():
            return mod._hook
        mod.set_axon_ntff_profile_hook = set_axon_ntff_profile_hook
        mod.get_axon_ntff_profile_hook = get_axon_ntff_profile_hook
        sys.modules["antenv.axon_hooks"] = mod
        import antenv
        antenv.axon_hooks = mod

        so_path = "/opt/axon/libaxon_pjrt.so"
        if os.path.exists(so_path):
            lib = ctypes.CDLL(so_path)
            if hasattr(lib, "axon_start_nrt_profile"):
                lib.axon_start_nrt_profile.argtypes = [
                    ctypes.POINTER(ctypes.c_int64), ctypes.c_size_t]
                lib.axon_start_nrt_profile.restype = ctypes.c_int64
                lib.axon_stop_nrt_profile.argtypes = [ctypes.c_char_p]
                lib.axon_stop_nrt_profile.restype = ctypes.c_int64

                @contextlib.contextmanager
                def _hook(output_dir, device_ids):
                    import jax
                    jax.devices()
                    if device_ids:
                        ids = (ctypes.c_int64 * len(device_ids))(*device_ids)
                        rc = lib.axon_start_nrt_profile(ids, len(device_ids))
                    else:
                        rc = lib.axon_start_nrt_profile(None, 0)
                    if rc != 0:
                        raise RuntimeError(f"axon_start_nrt_profile rc={rc}")
                    try:
                        yield
                    finally:
                        n = lib.axon_stop_nrt_profile(str(output_dir).encode())
                        print(f"profile: {n} ntff file(s) -> {output_dir}",
                              file=sys.stderr)

                set_axon_ntff_profile_hook(_hook)

    bass_utils.upload_artifacts = lambda tmpdir: f"local://{tmpdir}"


_NC_CACHE = {}


def _build_nc(S, S16, ROWS, pieces, probe=False):
    import concourse.bacc as bacc
    import concourse.tile as tile
    from concourse import mybir
    from contextlib import ExitStack

    cache_key = (S, S16, ROWS, tuple(pieces), probe)
    if cache_key in _NC_CACHE:
        return _NC_CACHE[cache_key]

    f32 = mybir.dt.float32
    AF = mybir.ActivationFunctionType
    ALU = mybir.AluOpType

    bf = mybir.dt.bfloat16
    nc = bacc.Bacc("TRN2", target_bir_lowering=False, debug=False,
                   num_devices=NCORES)
    d_c_t = nc.dram_tensor("d_c", (NGROUP * 9, S), bf, kind="ExternalInput")
    d_b_t = nc.dram_tensor("d_b", (128, S16), f32, kind="ExternalInput")
    vec_t = nc.dram_tensor("vecs", (128, 5), f32, kind="ExternalInput")
    cof_t = nc.dram_tensor("cofs", (NGROUP * 9, 128), f32, kind="ExternalInput")
    out_t = nc.dram_tensor("out", (NGROUP, 16, ROWS), f32, kind="ExternalOutput")

    import ml_dtypes
    nbf = ml_dtypes.bfloat16
    ind_w = np.zeros((16, 128, 128), nbf)
    for j in range(16):
        ind_w[j, (np.arange(128) // 16) * 16 + j, np.arange(128)] = 1.0
    ind_w_t = nc.inline_tensor(
        np.ascontiguousarray(ind_w.transpose(1, 0, 2)).reshape(128, 16 * 128),
        "ind_w")

    with tile.TileContext(nc) as tc, ExitStack() as ctx:
        cpool = ctx.enter_context(tc.tile_pool(name="consts", bufs=1))
        cof = cpool.tile([NGROUP * 9, 128], f32)
        nc.sync.dma_start(cof[:], cof_t.ap())
        lhsT_d = cpool.tile([NGROUP * 9, 128], bf)
        nc.vector.tensor_copy(lhsT_d[:], cof[:])
        lhsT_w = cpool.tile([128, 16 * 128], bf)
        nc.sync.dma_start(lhsT_w[:], ind_w_t.ap())
        vec = cpool.tile([128, 5], f32)
        nc.sync.dma_start(vec[:], vec_t.ap())
        dbt = cpool.tile([128, S16], f32)
        nc.sync.dma_start(dbt[:], d_b_t.ap())
        winb = cpool.tile([128, S16], f32)
        # win_sin = sin(pi/c * d + pi/2); win = 0.5*(1+win_sin)
        nc.scalar.activation(winb[:], dbt[:], AF.Sin,
                             bias=vec[:, 3:4], scale=vec[:, 2:3])
        w05 = cpool.tile([128, S16], f32)
        nc.vector.tensor_scalar(w05[:], winb[:], 0.5, 0.5,
                                op0=ALU.mult, op1=ALU.add)
        w05c = cpool.tile([128, S16], f32)
        nc.vector.tensor_scalar(w05c[:], w05[:], 1e-13, None, op0=ALU.max)
        lnw = cpool.tile([128, S16], f32)
        nc.scalar.activation(lnw[:], w05c[:], AF.Ln)
        nlnw = cpool.tile([128, S16], f32)
        nc.vector.tensor_scalar(nlnw[:], lnw[:], -1.0, None, op0=ALU.mult)
        wh = cpool.tile([128, S16], bf)
        nc.vector.tensor_copy(wh[:], nlnw[:])
        R = cpool.tile([128, ROWS], f32)

        dcp = ctx.enter_context(tc.tile_pool(name="dc", bufs=6))
        pdp = ctx.enter_context(tc.tile_pool(name="pd", bufs=4, space="PSUM"))
        hep = ctx.enter_context(tc.tile_pool(name="he", bufs=6))

        for p, (o, c, m, ro) in enumerate(pieces):
            psz = c * m
            j, s0 = p % 16, (p // 16) * PIECE
            dc = dcp.tile([NGROUP * 9, PIECE], bf, tag="dc")
            nc.gpsimd.dma_start(dc[:, :psz], d_c_t.ap()[:, o : o + psz])
            pd = pdp.tile([128, PIECE], f32, tag="pd")
            for h0 in range(0, psz, 512):
                h1 = min(h0 + 512, psz)
                nc.tensor.matmul(pd[:, h0:h1], lhsT_d[:], dc[:, h0:h1],
                                 start=True, stop=False)
                nc.tensor.matmul(pd[:, h0:h1], lhsT_w[:, j * 128 : (j + 1) * 128],
                                 wh[:, s0 + h0 : s0 + h1], start=False, stop=True)
            he = hep.tile([128, PIECE], bf, tag="he")
            nc.scalar.activation(he[:, :psz], pd[:, :psz], AF.Exp,
                                 bias=vec[:, 0:1], scale=vec[:, 1:2])
            red_in = he[:, :psz].rearrange("q (c m) -> q c m", m=m)
            nc.vector.tensor_reduce(R[:, ro : ro + c], red_in,
                                    axis=mybir.AxisListType.X, op=ALU.add)

        for g in range(NGROUP):
            nc.sync.dma_start(out_t.ap()[g], R[g * 16 : (g + 1) * 16, :])

        if probe:
            import concourse.bass as bass  # noqa
            ppool = ctx.enter_context(tc.tile_pool(name="probe", bufs=1))
            pdat = ppool.tile([128, 1024], f32)
            pidx = ppool.tile([128, 128], mybir.dt.int16)
            pout = ppool.tile([128, 2048], f32)
            nc.vector.memset(pdat[:], 1.0)
            nc.vector.memset(pidx[:].bitcast(f32), 0.0)
            with nc.named_scope("probe_apgather_512"):
                nc.gpsimd.ap_gather(pout[:, :512].rearrange("p (i d) -> p i d", d=1),
                                    pdat[:].rearrange("p (e d) -> p e d", d=1),
                                    pidx[:, :32], channels=128,
                                    num_elems=1024, d=1, num_idxs=512)
            with nc.named_scope("probe_apgather_2048"):
                nc.gpsimd.ap_gather(pout[:, :2048].rearrange("p (i d) -> p i d", d=1),
                                    pdat[:].rearrange("p (e d) -> p e d", d=1),
                                    pidx[:, :128], channels=128,
                                    num_elems=1024, d=1, num_idxs=2048)
            with nc.named_scope("probe_apgather_d16"):
                nc.gpsimd.ap_gather(pout[:, :2048].rearrange("p (i d) -> p i d", d=16),
                                    pdat[:].rearrange("p (e d) -> p e d", d=16),
                                    pidx[:, :8], channels=128,
                                    num_elems=64, d=16, num_idxs=128)

    nc.compile()
    _NC_CACHE[cache_key] = nc
    return nc


def kernel(**inputs):
    global PAD_D
    feat = np.asarray(inputs["feat"], np.float32)
    distances = np.asarray(inputs["distances"], np.float32)
    src = np.asarray(inputs["src"])
    dst = np.asarray(inputs["dst"])
    cutoffs = np.asarray(inputs["interaction_cutoffs"], np.float32)
    mu = np.asarray(inputs["rbf_kernel_means"], np.float32)
    scal = np.asarray(inputs["rbf_kernel_scaling"], np.float32)
    ftu = np.asarray(inputs["features_to_use"], np.float32)

    assert np.all(cutoffs == cutoffs[0]), "per-k cutoffs unsupported"
    cutoff = float(cutoffs[0])
    PAD_D = cutoff  # win(cutoff) == 0 kills padding contributions

    lay = _host_layout(feat, distances, src, dst, ftu)
    S, S16, ROWS, pieces = lay["S"], lay["S16"], lay["ROWS"], lay["pieces"]

    kk = np.arange(128) % 16
    # he = Exp(-(scal*q - 2*scal*mu*d + nlnw) - scal*mu^2) = gauss * win
    vecs = np.stack([
        (-scal[kk].astype(np.float64) * mu[kk].astype(np.float64) ** 2
         ).astype(np.float32),                       # Exp bias
        np.full(128, -1.0, np.float32),              # Exp scale
        np.full(128, -math.pi / cutoff, np.float32),  # Sin scale
        np.full(128, math.pi / 2, np.float32),       # Sin bias
        np.full(128, 0.0, np.float32),
    ], axis=1).astype(np.float32)
    import ml_dtypes
    nbf = ml_dtypes.bfloat16
    cd = (-2.0 * scal[kk].astype(np.float64) * mu[kk].astype(np.float64))
    ch = cd.astype(nbf).astype(np.float64)
    cl = (cd - ch).astype(nbf).astype(np.float64)
    cl2 = ((cd - ch) - cl).astype(nbf).astype(np.float32)
    sh = scal[kk].astype(nbf).astype(np.float32)
    cofs = np.zeros((NGROUP * 9, 128), np.float32)
    pp = np.arange(128)
    gg = pp // 16
    for r, coef in enumerate([ch, ch, ch, cl, cl, cl2, sh, sh, sh]):
        cofs[gg * 9 + r, pp] = coef.astype(np.float32)[pp]

    probe = bool(int(os.environ.get("KERNEL_PROBE", "0")))
    trace = bool(int(os.environ.get("KERNEL_TRACE", "0")))
    nc = _build_nc(S, S16, ROWS, pieces, probe=probe)

    from concourse import bass_utils
    if trace:
        _install_trace_shim(bass_utils)
    in_maps = [
        {"d_c": np.ascontiguousarray(lay["d_parts"][c]),
         "d_b": np.ascontiguousarray(lay["d_b"][c]),
         "vecs": vecs, "cofs": cofs}
        for c in range(NCORES)
    ]
    res = bass_utils.run_bass_kernel_spmd(
        nc, in_maps, core_ids=list(range(NCORES)), trace=trace,
        trace_cores=list(range(NCORES)) if trace else None,
    )
    LAST_RESULTS["res"] = res

    # gather/unshard: dev[core][g][k][row] -> out[v, t*K+k]
    dev = np.stack([r["out"] for r in res.results])  # (8, NGROUP, 16, ROWS)
    rows_all = dev.transpose(0, 1, 3, 2).reshape(NSTREAM, ROWS, K)
    seg_rows = rows_all[lay["strm_o"], lay["rowpos_o"]]  # (nseg, K)
    out = np.zeros((V * T, K), np.float32)
    seg_key = lay["seg_key"]
    if len(np.unique(seg_key)) == len(seg_key):
        out[seg_key] = seg_rows
    else:
        np.add.at(out, seg_key, seg_rows)
    return out.reshape(V, T * K).astype(np.float32)


if __name__ == "__main__":
    # smoke test with tiny random data through the same code paths
    rng = np.random.default_rng(0)
    nE, nV = 5000, 300
    feat = rng.integers(0, T, (nV, 1)).astype(np.float32)
    inputs = dict(
        feat=feat,
        distances=(rng.random((nE, 1)) * 12.0).astype(np.float32),
        src=rng.integers(0, nV, nE).astype(np.int32),
        dst=rng.integers(0, nV, nE).astype(np.int32),
        interaction_cutoffs=np.full(K, 12.0, np.float32),
        rbf_kernel_means=np.linspace(0, 12, K).astype(np.float32),
        rbf_kernel_scaling=np.ones(K, np.float32),
        features_to_use=np.arange(T, dtype=np.float32),
    )
    print(kernel(**inputs).sum())
